# revision 43
# baseline (speedup 1.0000x reference)
"""Trainium2 Bass kernel for GQA causal attention (nn_Attention_83623013253180).

Shapes: B=2, L=2048, D=1024, H=16 heads, G=2 kv-groups, HPG=8, DQK=DV=128.

Sharding (8 cores): core c -> (b = c//4, g = (c%4)//2, hh = c%2), each core
handles one batch, one kv group, and 4 of that group's 8 query heads.
Wq/Wk/Wv are column-sharded, Wo row-sharded; the out-proj all-reduce (sum of
4 partials per batch) is done on host after gather, along with + bo.

Per-core device kernel (all matmul operands fp16, PSUM fp32), organized as a
pipeline over 512-token chunks so DMA loads hide behind compute:
  chunk ch: load xk/xq/xv chunk -> kT/qT/V_aug projections -> causal
  attention for q-chunk ch (kv tiles 0..4*ch+3) -> out projection -> store.

  - inputs arrive pre-transposed from host: x[ch] = in_x[b].T chunk as
    [128, 8, 512] partition-tiled fp16
  - qT[h] = (Wq_h^T X^T)  [128 dqk, tok]   (lhsT=Wq tile, rhs=xT)
  - S^T tile = matmul(lhsT=kT slice, rhs=qT chunk) -> PSUM [128 kv, 512 q]
  - e = exp(S^T * scale) on ScalarE -> fp16 SBUF; causal 0/1 mask multiply
    on diagonal tiles (DVE)
  - ctx PSUM [q 128, 129] += matmul(lhsT=e slice, rhs=V_aug tile); V_aug has
    a ones column so col 128 accumulates the softmax denominator
  - normalize per-partition via reciprocal, PE-transpose ctx -> ctxT
  - out[q,1024] partial = sum_h matmul(lhsT=ctxT_h, rhs=Wo_h) -> DMA fp32
"""

import numpy as np

import concourse.bass as bass
import concourse.mybir as mybir
import concourse.tile as tile
from concourse import bacc
from concourse.bass_utils import run_bass_kernel_spmd

F16 = mybir.dt.float16
F32 = mybir.dt.float32

B, L, D = 2, 2048, 1024
H, G, HPG = 16, 2, 8
DQK = DV = 128
NHEAD = 4          # heads per core
NDT = D // 128     # 8 contraction tiles over input dim
NKV = L // 128     # 16 kv tiles
QC = 512           # q chunk width
NQC = L // QC      # 4 q chunks
NCORES = 8


def _build(scale_val: float) -> bass.Bass:
    nc = bacc.Bacc("TRN2", target_bir_lowering=False, debug=False, num_devices=NCORES)

    xq = nc.dram_tensor("xqT", [NQC, 128, NDT, QC], F16, kind="ExternalInput")
    xk = nc.dram_tensor("xkT", [NQC, 128, NDT, QC], F16, kind="ExternalInput")
    xv = nc.dram_tensor("xvT", [NQC, 128, NDT, QC], F16, kind="ExternalInput")
    wq = nc.dram_tensor("wq", [128, NDT, NHEAD * DQK], F16, kind="ExternalInput")
    wk = nc.dram_tensor("wk", [128, NDT, DQK], F16, kind="ExternalInput")
    wv = nc.dram_tensor("wv", [128, NDT, DV + 1], F16, kind="ExternalInput")
    wo = nc.dram_tensor("wo", [128, NHEAD, D], F16, kind="ExternalInput")
    bq = nc.dram_tensor("bq", [128, NHEAD], F32, kind="ExternalInput")
    bk = nc.dram_tensor("bk", [128, 1], F32, kind="ExternalInput")
    bvb = nc.dram_tensor("bvb", [128, DV + 1], F32, kind="ExternalInput")
    msk = nc.dram_tensor("msk", [128, 128], F16, kind="ExternalInput")
    idn = nc.dram_tensor("idn", [128, 128], F16, kind="ExternalInput")
    out = nc.dram_tensor("out", [L, D], F32, kind="ExternalOutput")

    with tile.TileContext(nc) as tc:
        with (
            tc.tile_pool(name="const", bufs=1) as cpool,
            tc.tile_pool(name="xbuf", bufs=1) as xpool,
            tc.tile_pool(name="qkv", bufs=1) as qkvpool,
            tc.tile_pool(name="work", bufs=8) as wpool,
            tc.tile_pool(name="masked", bufs=4) as mpool,
            tc.tile_pool(name="ctxt", bufs=3) as ctpool,
            tc.tile_pool(name="outb", bufs=4) as opool,
            tc.tile_pool(name="ps_a", bufs=3, space="PSUM") as ps_a,
            tc.tile_pool(name="ps_ctx", bufs=4, space="PSUM") as ps_ctx,
            tc.tile_pool(name="ps_tr", bufs=1, space="PSUM") as ps_tr,
        ):
            # weights/constants are loaded just before first use (ch==0) so
            # the first k-projection can start as early as possible
            wk_sb = cpool.tile([128, NDT, DQK], F16, tag="wk")
            bk_sb = cpool.tile([128, 1], F32, tag="bk")
            bq_sb = cpool.tile([128, NHEAD], F32, tag="bq")
            bvb_sb = cpool.tile([128, DV + 1], F32, tag="bvb")
            msk_sb = cpool.tile([128, 128], F16, tag="msk")
            idn_sb = cpool.tile([128, 128], F16, tag="idn")
            wq_sb = cpool.tile([128, NDT, NHEAD * DQK], F16, tag="wq")
            wv_sb = cpool.tile([128, NDT, DV + 1], F16, tag="wv")
            wo_sb = cpool.tile([128, NHEAD, D], F16, tag="wo")

            q_sb = qkvpool.tile([128, NHEAD, L], F16, tag="q")    # qT per head
            k_sb = qkvpool.tile([128, L], F16, tag="k")           # kT
            v_sb = qkvpool.tile([128, NKV, DV + 1], F16, tag="v")  # V_aug tiles

            xq_sb = xpool.tile([128, NQC, NDT, QC], F16, tag="xq")
            xk_sb = xpool.tile([128, NQC, NDT, QC], F16, tag="xk")
            xv_sb = xpool.tile([128, NQC, NDT, QC], F16, tag="xv")

            for ch in range(NQC):
                sl = slice(ch * QC, (ch + 1) * QC)

                # ---- load + project this chunk (k, then v, then q so PE
                # has fill work while the later loads stream in) ----
                if ch == 0:
                    nc.sync.dma_start(wk_sb[:], wk[:])
                    nc.sync.dma_start(bk_sb[:], bk[:])
                nc.sync.dma_start(xk_sb[:, ch], xk[ch])
                pk = ps_a.tile([128, 512], F32, tag="ps_a")
                for dt_i in range(NDT):
                    nc.tensor.matmul(
                        pk, wk_sb[:, dt_i, :], xk_sb[:, ch, dt_i, :],
                        start=(dt_i == 0), stop=(dt_i == NDT - 1),
                    )
                nc.vector.tensor_tensor(
                    k_sb[:, sl], pk, bk_sb[:].to_broadcast((128, 512)),
                    mybir.AluOpType.add,
                )

                if ch == 0:
                    nc.sync.dma_start(wv_sb[:], wv[:])
                    nc.sync.dma_start(bvb_sb[:], bvb[:])
                    nc.sync.dma_start(msk_sb[:], msk[:])
                    nc.sync.dma_start(idn_sb[:], idn[:])
                nc.sync.dma_start(xv_sb[:, ch], xv[ch])
                for kvs in range(4):
                    kv = ch * 4 + kvs
                    pv = ps_a.tile([128, DV + 1], F32, tag="ps_a")
                    for dt_i in range(NDT):
                        nc.tensor.matmul(
                            pv, xv_sb[:, ch, dt_i, kvs * 128:(kvs + 1) * 128],
                            wv_sb[:, dt_i, :],
                            start=(dt_i == 0), stop=(dt_i == NDT - 1),
                        )
                    nc.vector.tensor_tensor(
                        v_sb[:, kv, :], pv, bvb_sb[:], mybir.AluOpType.add
                    )

                if ch == 0:
                    nc.sync.dma_start(wq_sb[:], wq[:])
                    nc.sync.dma_start(bq_sb[:], bq[:])
                nc.sync.dma_start(xq_sb[:, ch], xq[ch])
                for hi in range(NHEAD):
                    pq = ps_a.tile([128, 512], F32, tag="ps_a")
                    for dt_i in range(NDT):
                        nc.tensor.matmul(
                            pq,
                            wq_sb[:, dt_i, hi * DQK:(hi + 1) * DQK],
                            xq_sb[:, ch, dt_i, :],
                            start=(dt_i == 0), stop=(dt_i == NDT - 1),
                        )
                    nc.vector.tensor_tensor(
                        q_sb[:, hi, sl], pq,
                        bq_sb[:, hi:hi + 1].to_broadcast((128, 512)),
                        mybir.AluOpType.add,
                    )

                # ---- attention for q chunk ch (kv tiles 0..4*ch+3) ----
                qc = ch
                ctxT = ctpool.tile([128, NHEAD, 4, 128], F16, tag="ctxT")
                for hi in range(NHEAD):
                    nkv_c = 4 * qc + 4
                    ctx_ps = [
                        ps_ctx.tile([128, DV + 1], F32, tag="ctx",
                                    name=f"ctx_{qc}_{hi}_{j}")
                        for j in range(4)
                    ]
                    for kv in range(nkv_c):
                        t = kv - 4 * qc
                        # causal: q columns below kv tile start are all
                        # masked -> shrink score/exp width to the live part
                        qoff = max(t, 0) * 128
                        w = QC - qoff
                        s_ps = ps_a.tile([128, QC], F32, tag="ps_a")
                        nc.tensor.matmul(
                            s_ps[:, :w],
                            k_sb[:, kv * 128:(kv + 1) * 128],
                            q_sb[:, hi, qc * QC + qoff:(qc + 1) * QC],
                            start=True, stop=True,
                        )
                        e_sb = wpool.tile([128, QC], F16, tag="e")
                        nc.scalar.activation(
                            e_sb[:, :w], s_ps[:, :w],
                            mybir.ActivationFunctionType.Exp,
                            bias=0.0, scale=scale_val,
                        )
                        if t >= 0:
                            # only the leading 128 block straddles the
                            # diagonal; later blocks are fully allowed
                            em_sb = mpool.tile([128, 128], F16, tag="em")
                            nc.vector.tensor_tensor(
                                em_sb[:], e_sb[:, 0:128], msk_sb[:],
                                mybir.AluOpType.mult,
                            )
                        for j in range(4):
                            if kv > 4 * qc + j:
                                continue
                            if j == max(t, 0) and t >= 0:
                                e_use = em_sb[:, 0:128]
                            else:
                                e_use = e_sb[:, j * 128 - qoff:
                                             (j + 1) * 128 - qoff]
                            nc.tensor.matmul(
                                ctx_ps[j],
                                e_use,
                                v_sb[:, kv, :],
                                start=(kv == 0), stop=(kv == 4 * qc + j),
                            )
                    for j in range(4):
                        rcp = wpool.tile([128, 1], F32, tag="rcp")
                        nc.vector.reciprocal(rcp[:], ctx_ps[j][:, DV:DV + 1])
                        ctxn = wpool.tile([128, 128], F16, tag="ctxn")
                        nc.vector.tensor_tensor(
                            ctxn[:], ctx_ps[j][:, 0:DV],
                            rcp[:].to_broadcast((128, DV)),
                            mybir.AluOpType.mult,
                        )
                        tr_ps = ps_tr.tile([128, 128], F16, tag="tr")
                        nc.tensor.transpose(tr_ps, ctxn[:], idn_sb[:])
                        nc.vector.tensor_copy(ctxT[:, hi, j, :], tr_ps)

                # ---- out projection for this q chunk ----
                if ch == 0:
                    nc.sync.dma_start(wo_sb[:], wo[:])
                for j in range(4):
                    o_sb = opool.tile([128, D], F32, tag="o")
                    for nch in range(2):
                        po = ps_a.tile([128, 512], F32, tag="ps_a")
                        for hi in range(NHEAD):
                            nc.tensor.matmul(
                                po,
                                ctxT[:, hi, j, :],
                                wo_sb[:, hi, nch * 512:(nch + 1) * 512],
                                start=(hi == 0), stop=(hi == NHEAD - 1),
                            )
                        nc.vector.tensor_copy(
                            o_sb[:, nch * 512:(nch + 1) * 512], po
                        )
                        qt = qc * 4 + j
                        nc.sync.dma_start(
                            out[qt * 128:(qt + 1) * 128,
                                nch * 512:(nch + 1) * 512],
                            o_sb[:, nch * 512:(nch + 1) * 512],
                        )

    nc.finalize()
    return nc


_NC_CACHE: dict[float, bass.Bass] = {}


def _get_nc(scale_val: float) -> bass.Bass:
    if scale_val not in _NC_CACHE:
        _NC_CACHE[scale_val] = _build(scale_val)
    return _NC_CACHE[scale_val]


def _chunk_tile(a: np.ndarray) -> np.ndarray:
    """[K, F] -> [F//QC, 128, K//128, QC] chunk-major partition-tiled fp16."""
    k, f = a.shape
    b = a.reshape(k // 128, 128, f // QC, QC)          # [po, pi, ch, qc]
    return np.ascontiguousarray(
        b.transpose(2, 1, 0, 3).astype(np.float16)     # [ch, pi, po, qc]
    )


def _part_tile(a: np.ndarray) -> np.ndarray:
    """[K, F] -> [128, K//128, F] partition-tiled fp16 contiguous."""
    k, f = a.shape
    return np.ascontiguousarray(
        a.reshape(k // 128, 128, f).transpose(1, 0, 2).astype(np.float16)
    )


def run(inputs: dict, trace: bool = False):
    in_q = np.asarray(inputs["in_q"], np.float32)
    in_k = np.asarray(inputs["in_k"], np.float32)
    in_v = np.asarray(inputs["in_v"], np.float32)
    Wq = np.asarray(inputs["Wq"], np.float32)
    Wk = np.asarray(inputs["Wk"], np.float32)
    Wv = np.asarray(inputs["Wv"], np.float32)
    Wo = np.asarray(inputs["Wo"], np.float32)
    bq = np.asarray(inputs["bq"], np.float32)
    bk = np.asarray(inputs["bk"], np.float32)
    bv = np.asarray(inputs["bv"], np.float32)
    bo = np.asarray(inputs["bo"], np.float32)
    qes = float(np.asarray(inputs["q_extra_scale"], np.float32).reshape(-1)[0])

    scale_val = qes / float(np.sqrt(DQK))
    nc = _get_nc(scale_val)

    # triangular mask for the single diagonal 128x128 block
    ii = np.arange(128)[:, None]
    jj = np.arange(128)[None, :]
    masks = (jj >= ii).astype(np.float16)  # [128, 128], 1 where q >= kv
    idn = np.eye(128, dtype=np.float16)

    in_maps = []
    for c in range(NCORES):
        b, g, hh = c // 4, (c % 4) // 2, c % 2
        h0 = g * HPG + hh * NHEAD
        wv_aug = np.concatenate(
            [Wv[:, g * DV:(g + 1) * DV], np.zeros((D, 1), np.float32)], axis=1
        )
        bv_aug = np.concatenate([bv[g * DV:(g + 1) * DV], [1.0]]).astype(np.float32)
        wo_slice = Wo[h0 * DV:(h0 + NHEAD) * DV, :]  # [512, 1024]
        in_maps.append({
            "xqT": _chunk_tile(in_q[b].T),
            "xkT": _chunk_tile(in_k[b].T),
            "xvT": _chunk_tile(in_v[b].T),
            "wq": _part_tile(Wq[:, h0 * DQK:(h0 + NHEAD) * DQK]),
            "wk": _part_tile(Wk[:, g * DQK:(g + 1) * DQK]),
            "wv": _part_tile(wv_aug),
            "wo": np.ascontiguousarray(
                wo_slice.reshape(NHEAD, DV, D).transpose(1, 0, 2).astype(np.float16)
            ),
            "bq": np.ascontiguousarray(
                bq[h0 * DQK:(h0 + NHEAD) * DQK].reshape(NHEAD, DQK).T.astype(np.float32)
            ),
            "bk": bk[g * DQK:(g + 1) * DQK].reshape(DQK, 1).astype(np.float32),
            "bvb": np.ascontiguousarray(
                np.broadcast_to(bv_aug, (128, DV + 1)).astype(np.float32)
            ),
            "msk": masks,
            "idn": idn,
        })

    res = run_bass_kernel_spmd(
        nc, in_maps, core_ids=list(range(NCORES)), trace=trace
    )

    out_full = np.zeros((B, L, D), np.float32)
    for c in range(NCORES):
        out_full[c // 4] += np.asarray(res.results[c]["out"], np.float32)
    out_full += bo
    return out_full, res.exec_time_ns


def kernel(**inputs) -> np.ndarray:
    out, _ = run(inputs, trace=False)
    return out
